# revision 11
# baseline (speedup 1.0000x reference)
"""Causal multi-head self-attention with RoPE on 8 NeuronCores.

Sharding (hardcoded): core c -> batch b = c // 2, head-group hg = c % 2.
Each core:
  - projects its batch's x with column-sharded WQ/WK/WV (8 heads = 512 dims),
  - applies RoPE (host-precomputed cos/sin tables, adjacent-pair swap done
    on-chip with stream_shuffle),
  - runs causal attention for its 8 heads in transposed layout
    (S^T = [k, q]; softmax denominator comes free from a ones-column
    appended to V; normalization is broadcast via a K=1 matmul),
  - applies the row-sharded WO projection -> partial [T, D] output.
Host sums the two partials per batch (the "all-reduce after WO").
"""

import numpy as np
import ml_dtypes

B, T, D, H = 4, 2048, 1024, 16
DK = 64
HLOC = 8          # heads per core
E = HLOC * DK     # 512, local projection width
NCORES = 8
THETA = 10000.0

_BF16 = ml_dtypes.bfloat16

_cache = {}


def _build(t=T, hloc=HLOC, d=D, reps=1):
    from contextlib import ExitStack

    import concourse.bacc as bacc
    import concourse.bass as bass  # noqa: F401
    import concourse.mybir as mybir
    import concourse.tile as tile

    f32 = mybir.dt.float32
    bf16 = mybir.dt.bfloat16
    Exp = mybir.ActivationFunctionType.Exp

    e = hloc * DK
    npair = hloc // 2       # head-pair tiles in QT/KT/OT
    dsub = d // 128         # contraction subtiles for projections
    tq = t // 512           # 512-wide q chunks
    tk = t // 128           # 128-wide k tiles
    ep = e // 128           # output-partition tiles for Q/K (= npair)
    swap_mask = [i ^ 1 for i in range(32)]

    nc = bacc.Bacc("TRN2", target_bir_lowering=False, debug=False)

    xT = nc.declare_dram_parameter("xT", [d, t], bf16, False).ap()
    wqT = nc.declare_dram_parameter("wqT", [d, e], bf16, False).ap()
    wkT = nc.declare_dram_parameter("wkT", [d, e], bf16, False).ap()
    wvT = nc.declare_dram_parameter("wvT", [d, e], bf16, False).ap()
    woT = nc.declare_dram_parameter("woT", [e, d], bf16, False).ap()
    cosT = nc.declare_dram_parameter("cosT", [128, t], f32, False).ap()
    sinT = nc.declare_dram_parameter("sinT", [128, t], f32, False).ap()
    trim = nc.declare_dram_parameter("trim", [128, 128], bf16, False).ap()
    y = nc.declare_dram_parameter("y", [t, d], f32, True).ap()
    scr = nc.dram_tensor("scr", [hloc, t], bf16)  # denom-recip bounce for bcast

    with tile.TileContext(nc) as tc:
        with ExitStack() as ctx:
            const = ctx.enter_context(tc.tile_pool(name="const", bufs=1))
            ptpool = ctx.enter_context(tc.tile_pool(name="ptp", bufs=3))
            normp = ctx.enter_context(tc.tile_pool(name="normp", bufs=2))
            ysbp = ctx.enter_context(tc.tile_pool(name="ysbp", bufs=3))

            wq_sb = const.tile([128, dsub, e], bf16)
            wk_sb = const.tile([128, dsub, e], bf16)
            wv_sb = const.tile([128, dsub, e], bf16)
            wo_sb = const.tile([128, e // 128, d], bf16)
            trim_sb = const.tile([128, 128], bf16)
            qt_sb = const.tile([128, npair, t], bf16)
            kt_sb = const.tile([128, npair, t], bf16)
            v_sb = const.tile([128, tk, hloc, DK + 1], bf16)
            ot_sb = const.tile([128, npair, t], bf16)

            nc.sync.dma_start(wq_sb, wqT.rearrange("(n p) e -> p n e", p=128))
            nc.sync.dma_start(wk_sb, wkT.rearrange("(n p) e -> p n e", p=128))
            nc.sync.dma_start(wv_sb, wvT.rearrange("(n p) e -> p n e", p=128))
            nc.sync.dma_start(wo_sb, woT.rearrange("(n p) d -> p n d", p=128))
            nc.sync.dma_start(trim_sb, trim)
            nc.vector.memset(v_sb[:, :, :, DK : DK + 1], 1.0)

            xt_sb = const.tile([128, dsub, t], bf16)
            cos_sb = const.tile([128, t], f32)
            sin_sb = const.tile([128, t], f32)
            nc.sync.dma_start(xt_sb, xT.rearrange("(n p) t -> p n t", p=128))
            nc.sync.dma_start(cos_sb, cosT)
            nc.sync.dma_start(sin_sb, sinT)

            for _rep in range(reps):
                # ------------- Phase 1: QKV projections + RoPE -------------
                with ExitStack() as c1:
                    rope = c1.enter_context(tc.tile_pool(name="rope", bufs=2))
                    ppsum = c1.enter_context(
                        tc.tile_pool(name="ppsum", bufs=2, space="PSUM")
                    )

                    for wsb, dst in ((wq_sb, qt_sb), (wk_sb, kt_sb)):
                        for ie in range(ep):
                            q_ps = ppsum.tile([128, t], f32, tag="proj")
                            for ds_ in range(dsub):
                                for jt in range(tq):
                                    nc.tensor.matmul(
                                        q_ps[:, jt * 512 : (jt + 1) * 512],
                                        lhsT=wsb[:, ds_, ie * 128 : (ie + 1) * 128],
                                        rhs=xt_sb[:, ds_, jt * 512 : (jt + 1) * 512],
                                        start=(ds_ == 0),
                                        stop=(ds_ == dsub - 1),
                                    )
                            # RoPE: out = cos * q + sinS * pairswap(q)
                            sw = rope.tile([128, t], f32, tag="sw")
                            nc.vector.stream_shuffle(sw, q_ps[:, :], mask=swap_mask)
                            nc.vector.tensor_mul(dst[:, ie, :], q_ps[:, :], cos_sb)
                            nc.gpsimd.tensor_mul(sw, sw, sin_sb)
                            nc.gpsimd.tensor_add(dst[:, ie, :], dst[:, ie, :], sw)

                    for it in range(tk):
                        v_ps = ppsum.tile([128, 512], f32, tag="proj")
                        nfree = min(512, e)
                        for ds_ in range(dsub):
                            nc.tensor.matmul(
                                v_ps[:, :nfree],
                                lhsT=xt_sb[:, ds_, it * 128 : (it + 1) * 128],
                                rhs=wv_sb[:, ds_, :nfree],
                                start=(ds_ == 0),
                                stop=(ds_ == dsub - 1),
                            )
                        nc.vector.tensor_copy(
                            v_sb[:, it, :, 0:DK],
                            v_ps[:, :nfree].rearrange("p (h k) -> p h k", h=hloc),
                        )

                # ------------- Phase 2: attention per head -------------
                with ExitStack() as c2:
                    spsum = c2.enter_context(
                        tc.tile_pool(name="spsum", bufs=2, space="PSUM")
                    )
                    opsum = c2.enter_context(
                        tc.tile_pool(name="opsum", bufs=1, space="PSUM")
                    )
                    for h in range(hloc):
                        pr, h2 = h // 2, h % 2
                        po = 64 * h2
                        o_ps = opsum.tile([128, t], f32, tag="o")
                        for i in range(tk):
                            j0, m = i // 4, i % 4
                            pt = ptpool.tile([128, t], bf16, tag="pt")
                            # q-subtiles processed in pairs -> [128, 1024]
                            # psum tiles, 2 bufs => exp overlaps next matmuls
                            for jh in range(j0 - j0 % 2, tq, 2):
                                js = [j for j in (jh, jh + 1) if j0 <= j < tq]
                                w = 512 * len(js)
                                s_ps = spsum.tile([128, 1024], f32, tag="s")
                                for idx, j in enumerate(js):
                                    clo = idx * 512 + (m * 128 if j == j0 else 0)
                                    qlo = j * 512 + (m * 128 if j == j0 else 0)
                                    nc.tensor.matmul(
                                        s_ps[:, clo : (idx + 1) * 512],
                                        lhsT=kt_sb[
                                            po : po + 64, pr, i * 128 : (i + 1) * 128
                                        ],
                                        rhs=qt_sb[
                                            po : po + 64, pr, qlo : (j + 1) * 512
                                        ],
                                        start=True,
                                        stop=True,
                                    )
                                off = m * 128 if js[0] == j0 else 0
                                nc.scalar.activation(
                                    pt[:, js[0] * 512 + off : (js[-1] + 1) * 512],
                                    s_ps[:, off : w],
                                    Exp,
                                    scale=DK ** (-0.5),
                                )
                            lo = j0 * 512 + m * 128
                            if m > 0:
                                nc.gpsimd.memset(pt[:, j0 * 512 : lo], 0.0)
                            nc.gpsimd.tensor_mul(
                                pt[:, lo : lo + 128], pt[:, lo : lo + 128], trim_sb
                            )
                            for j in range(j0, tq):
                                nc.tensor.matmul(
                                    o_ps[0:65, j * 512 : (j + 1) * 512],
                                    lhsT=v_sb[:, i, h, :],
                                    rhs=pt[:, j * 512 : (j + 1) * 512],
                                    start=(i == 0),
                                    stop=(i == 4 * j + 3),
                                )
                        # normalize: O[dk, q] / denom[q] (row 64 of o_ps).
                        # Broadcast recip across partitions via a DRAM bounce
                        # with a stride-0 partition read.
                        recip = normp.tile([1, t], bf16, tag="recip")
                        with nc.allow_low_precision(reason="softmax recip bf16"):
                            nc.vector.reciprocal(recip[0:1, :], o_ps[64:65, 0:t])
                        nc.sync.dma_start(scr[h : h + 1, :], recip[0:1, :])
                        bc_sb = normp.tile([64, t], bf16, tag="bc")
                        scr_bcast = bass.AP(
                            tensor=scr.ap().tensor,
                            offset=h * t,
                            ap=[[0, 64], [1, t]],
                        )
                        nc.sync.dma_start(bc_sb, scr_bcast)
                        nc.vector.tensor_mul(
                            ot_sb[po : po + 64, pr, :], o_ps[0:64, 0:t], bc_sb
                        )

                # ------------- Phase 3: WO projection -------------
                with ExitStack() as c3:
                    ypsum = c3.enter_context(
                        tc.tile_pool(name="ypsum", bufs=4, space="PSUM")
                    )
                    nech = d // 512
                    for it in range(tk):
                        y_sb = ysbp.tile([128, d], f32, tag="ysb")
                        for ec in range(nech):
                            y_ps = ypsum.tile([128, 512], f32, tag="y")
                            for dp in range(e // 128):
                                nc.tensor.matmul(
                                    y_ps,
                                    lhsT=ot_sb[:, dp, it * 128 : (it + 1) * 128],
                                    rhs=wo_sb[:, dp, ec * 512 : (ec + 1) * 512],
                                    start=(dp == 0),
                                    stop=(dp == e // 128 - 1),
                                )
                            nc.vector.tensor_copy(
                                y_sb[:, ec * 512 : (ec + 1) * 512], y_ps
                            )
                        nc.sync.dma_start(y[it * 128 : (it + 1) * 128, :], y_sb)

    nc.compile()
    return nc


def _get_nc():
    if "nc" not in _cache:
        _cache["nc"] = _build()
    return _cache["nc"]


def _host_tables(positions):
    """cos/sin RoPE tables laid out for the on-chip [128, T] tiles."""
    pos = np.asarray(positions, np.float32)  # [t]
    inv = 1.0 / THETA ** (
        (2.0 * np.arange(1, DK // 2 + 1, dtype=np.float32) - 2.0) / DK
    )  # [32]
    ang = pos[None, :] * inv[:, None]  # [32, t]
    c32 = np.cos(ang)
    s32 = np.sin(ang)
    rows = np.arange(128)
    dloc = rows % DK
    fidx = dloc // 2
    sign = np.where(dloc % 2 == 0, -1.0, 1.0).astype(np.float32)
    cosT = c32[fidx, :]
    sinT = sign[:, None] * s32[fidx, :]
    return np.ascontiguousarray(cosT), np.ascontiguousarray(sinT)


def _make_in_maps(inputs):
    x = np.asarray(inputs["x"], np.float32)
    token_positions = np.asarray(inputs["token_positions"])
    WQ = np.asarray(inputs["WQ"], np.float32)
    WK = np.asarray(inputs["WK"], np.float32)
    WV = np.asarray(inputs["WV"], np.float32)
    WO = np.asarray(inputs["WO"], np.float32)
    trimask = np.triu(np.ones((128, 128), np.float32)).astype(_BF16)

    in_maps = []
    for c in range(NCORES):
        b, hg = c // 2, c % 2
        sl = slice(hg * E, (hg + 1) * E)
        cosT, sinT = _host_tables(token_positions[b])
        in_maps.append(
            {
                "xT": np.ascontiguousarray(x[b].T).astype(_BF16),
                "wqT": np.ascontiguousarray(WQ[sl, :].T).astype(_BF16),
                "wkT": np.ascontiguousarray(WK[sl, :].T).astype(_BF16),
                "wvT": np.ascontiguousarray(WV[sl, :].T).astype(_BF16),
                "woT": np.ascontiguousarray(WO[:, sl].T).astype(_BF16),
                "cosT": cosT,
                "sinT": sinT,
                "trim": trimask,
            }
        )
    return in_maps


def kernel(x, token_positions, WQ, WK, WV, WO):
    from concourse.bass_utils import run_bass_kernel_spmd

    nc = _get_nc()
    in_maps = _make_in_maps(
        {
            "x": x,
            "token_positions": token_positions,
            "WQ": WQ,
            "WK": WK,
            "WV": WV,
            "WO": WO,
        }
    )
    res = run_bass_kernel_spmd(nc, in_maps, core_ids=list(range(NCORES)))
    out = np.empty((B, T, D), np.float32)
    for b in range(B):
        out[b] = res.results[2 * b]["y"] + res.results[2 * b + 1]["y"]
    return out


# revision 13
# speedup vs baseline: 1.1102x; 1.1102x over previous
"""Causal multi-head self-attention with RoPE on 8 NeuronCores.

Sharding (hardcoded): core c -> batch b = c // 2, head-group hg = c % 2.
Each core:
  - projects its batch's x with column-sharded WQ/WK/WV (8 heads = 512 dims),
  - applies RoPE (host-precomputed cos/sin tables, adjacent-pair swap done
    on-chip with stream_shuffle),
  - runs causal attention for its 8 heads in transposed layout
    (S^T = [k, q]; softmax denominator comes free from a ones-column
    appended to V; normalization is broadcast via a K=1 matmul),
  - applies the row-sharded WO projection -> partial [T, D] output.
Host sums the two partials per batch (the "all-reduce after WO").
"""

import numpy as np
import ml_dtypes

B, T, D, H = 4, 2048, 1024, 16
DK = 64
HLOC = 8          # heads per core
E = HLOC * DK     # 512, local projection width
NCORES = 8
THETA = 10000.0

_BF16 = ml_dtypes.bfloat16

_cache = {}


def _build(t=T, hloc=HLOC, d=D, reps=1):
    from contextlib import ExitStack

    import concourse.bacc as bacc
    import concourse.bass as bass  # noqa: F401
    import concourse.mybir as mybir
    import concourse.tile as tile

    f32 = mybir.dt.float32
    bf16 = mybir.dt.bfloat16
    Exp = mybir.ActivationFunctionType.Exp

    e = hloc * DK
    npair = hloc // 2       # head-pair tiles in QT/KT/OT
    dsub = d // 128         # contraction subtiles for projections
    tq = t // 512           # 512-wide q chunks
    tk = t // 128           # 128-wide k tiles
    ep = e // 128           # output-partition tiles for Q/K (= npair)
    swap_mask = [i ^ 1 for i in range(32)]
    e_v = min(512, e)

    nc = bacc.Bacc("TRN2", target_bir_lowering=False, debug=False)

    xT = nc.declare_dram_parameter("xT", [d, t], bf16, False).ap()
    wqT = nc.declare_dram_parameter("wqT", [d, e], bf16, False).ap()
    wkT = nc.declare_dram_parameter("wkT", [d, e], bf16, False).ap()
    wvT = nc.declare_dram_parameter("wvT", [d, e], bf16, False).ap()
    woT = nc.declare_dram_parameter("woT", [e, d], bf16, False).ap()
    cosT = nc.declare_dram_parameter("cosT", [128, t], f32, False).ap()
    sinT = nc.declare_dram_parameter("sinT", [128, t], f32, False).ap()
    trim = nc.declare_dram_parameter("trim", [128, 128], bf16, False).ap()
    y = nc.declare_dram_parameter("y", [t, d], f32, True).ap()
    scr = nc.dram_tensor("scr", [hloc, t], bf16)  # denom-recip bounce for bcast

    with tile.TileContext(nc) as tc:
        with ExitStack() as ctx:
            const = ctx.enter_context(tc.tile_pool(name="const", bufs=1))
            ptpool = ctx.enter_context(tc.tile_pool(name="ptp", bufs=3))
            normp = ctx.enter_context(tc.tile_pool(name="normp", bufs=2))
            ysbp = ctx.enter_context(tc.tile_pool(name="ysbp", bufs=3))

            wq_sb = const.tile([128, dsub, e], bf16)
            wk_sb = const.tile([128, dsub, e], bf16)
            wv_sb = const.tile([128, dsub, e], bf16)
            wo_sb = const.tile([128, e // 128, d], bf16)
            trim_sb = const.tile([128, 128], bf16)
            qt_sb = const.tile([128, npair, t], bf16)
            kt_sb = const.tile([128, npair, t], bf16)
            v_sb = const.tile([128, tk, hloc, DK + 1], bf16)
            ot_sb = const.tile([128, npair, t], bf16)

            nc.sync.dma_start(wq_sb, wqT.rearrange("(n p) e -> p n e", p=128))
            nc.sync.dma_start(wk_sb, wkT.rearrange("(n p) e -> p n e", p=128))
            nc.sync.dma_start(wv_sb, wvT.rearrange("(n p) e -> p n e", p=128))
            nc.sync.dma_start(wo_sb, woT.rearrange("(n p) d -> p n d", p=128))
            nc.sync.dma_start(trim_sb, trim)
            nc.vector.memset(v_sb[:, :, :, DK : DK + 1], 1.0)

            xt_sb = const.tile([128, dsub, t], bf16)
            cos_sb = const.tile([128, t], f32)
            sin_sb = const.tile([128, t], f32)
            nc.sync.dma_start(xt_sb, xT.rearrange("(n p) t -> p n t", p=128))
            nc.sync.dma_start(cos_sb, cosT)
            nc.sync.dma_start(sin_sb, sinT)

            for _rep in range(reps):
                with ExitStack() as c1:
                    rope = c1.enter_context(tc.tile_pool(name="rope", bufs=3))
                    ppsum = c1.enter_context(
                        tc.tile_pool(name="ppsum", bufs=1, space="PSUM")
                    )
                    spsum = c1.enter_context(
                        tc.tile_pool(name="spsum", bufs=2, space="PSUM")
                    )
                    opsum = c1.enter_context(
                        tc.tile_pool(name="opsum", bufs=2, space="PSUM")
                    )

                    # ---- V projection (psum via the shared "s" slots) ----
                    for it in range(tk):
                        v_ps = spsum.tile([128, 1024], f32, tag="s")
                        for ds_ in range(dsub):
                            nc.tensor.matmul(
                                v_ps[:, :e_v],
                                lhsT=xt_sb[:, ds_, it * 128 : (it + 1) * 128],
                                rhs=wv_sb[:, ds_, :e_v],
                                start=(ds_ == 0),
                                stop=(ds_ == dsub - 1),
                            )
                        nc.vector.tensor_copy(
                            v_sb[:, it, :, 0:DK],
                            v_ps[:, :e_v].rearrange("p (h k) -> p h k", h=hloc),
                        )

                    ph = min(1024, t)          # projection t-chunk
                    nh = t // ph               # chunks per e-tile
                    for pr in range(npair):
                        # ---- Q/K projection + RoPE for this head pair ----
                        for wsb, dst in ((wq_sb, qt_sb), (wk_sb, kt_sb)):
                            for ch in range(nh):
                                q_ps = ppsum.tile([128, ph], f32, tag="proj")
                                c0 = ch * ph
                                for ds_ in range(dsub):
                                    for jt in range(ph // 512):
                                        lo = c0 + jt * 512
                                        nc.tensor.matmul(
                                            q_ps[:, jt * 512 : (jt + 1) * 512],
                                            lhsT=wsb[:, ds_, pr * 128 : (pr + 1) * 128],
                                            rhs=xt_sb[:, ds_, lo : lo + 512],
                                            start=(ds_ == 0),
                                            stop=(ds_ == dsub - 1),
                                        )
                                # evict raw proj to sbuf, then RoPE from sbuf
                                raw = rope.tile([128, ph], f32, tag="raw")
                                nc.vector.tensor_copy(raw, q_ps[:, :])
                                sw = rope.tile([128, ph], f32, tag="sw")
                                nc.vector.stream_shuffle(sw, raw, mask=swap_mask)
                                nc.vector.tensor_mul(
                                    dst[:, pr, c0 : c0 + ph],
                                    raw,
                                    cos_sb[:, c0 : c0 + ph],
                                )
                                nc.gpsimd.tensor_mul(sw, sw, sin_sb[:, c0 : c0 + ph])
                                nc.gpsimd.tensor_add(
                                    dst[:, pr, c0 : c0 + ph],
                                    dst[:, pr, c0 : c0 + ph],
                                    sw,
                                )

                        # ---- attention for heads (2pr, 2pr+1), q-subtiles ----
                        hA, hB = 2 * pr, 2 * pr + 1
                        for j in range(tq):
                            o_a = opsum.tile([128, 512], f32, tag="o")
                            o_b = opsum.tile([128, 512], f32, tag="o")
                            for i in range(4 * j + 4):
                                off = 128 * (i % 4) if i // 4 == j else 0
                                qlo = j * 512
                                s_ps = spsum.tile([128, 1024], f32, tag="s")
                                nc.tensor.matmul(
                                    s_ps[:, 0:512],
                                    lhsT=kt_sb[0:64, pr, i * 128 : (i + 1) * 128],
                                    rhs=qt_sb[0:64, pr, qlo : (j + 1) * 512],
                                    start=True,
                                    stop=True,
                                )
                                nc.tensor.matmul(
                                    s_ps[:, 512:1024],
                                    lhsT=kt_sb[64:128, pr, i * 128 : (i + 1) * 128],
                                    rhs=qt_sb[64:128, pr, qlo : (j + 1) * 512],
                                    start=True,
                                    stop=True,
                                )
                                pt = ptpool.tile([128, 1024], bf16, tag="pt")
                                # full-width exp: stale cols land in regions
                                # that are memset/masked below
                                nc.scalar.activation(
                                    pt[:, :], s_ps[:, :], Exp, scale=DK ** (-0.5)
                                )
                                if off:
                                    nc.gpsimd.memset(pt[:, 0:off], 0.0)
                                    nc.gpsimd.memset(pt[:, 512 : 512 + off], 0.0)
                                if i // 4 == j:
                                    nc.gpsimd.tensor_mul(
                                        pt[:, off : off + 128],
                                        pt[:, off : off + 128],
                                        trim_sb,
                                    )
                                    nc.gpsimd.tensor_mul(
                                        pt[:, 512 + off : 512 + off + 128],
                                        pt[:, 512 + off : 512 + off + 128],
                                        trim_sb,
                                    )
                                nc.tensor.matmul(
                                    o_a[0:65, :],
                                    lhsT=v_sb[:, i, hA, :],
                                    rhs=pt[:, 0:512],
                                    start=(i == 0),
                                    stop=(i == 4 * j + 3),
                                )
                                nc.tensor.matmul(
                                    o_b[0:65, :],
                                    lhsT=v_sb[:, i, hB, :],
                                    rhs=pt[:, 512:1024],
                                    start=(i == 0),
                                    stop=(i == 4 * j + 3),
                                )
                            # normalize both heads for this q-subtile
                            for hx, o_ps, po in ((hA, o_a, 0), (hB, o_b, 64)):
                                recip = normp.tile([1, 512], bf16, tag="recip")
                                with nc.allow_low_precision(
                                    reason="softmax recip bf16"
                                ):
                                    nc.vector.reciprocal(
                                        recip[0:1, :], o_ps[64:65, 0:512]
                                    )
                                nc.sync.dma_start(
                                    scr[hx : hx + 1, j * 512 : (j + 1) * 512],
                                    recip[0:1, :],
                                )
                                bc_sb = normp.tile([64, 512], bf16, tag="bc")
                                scr_bcast = bass.AP(
                                    tensor=scr.ap().tensor,
                                    offset=hx * t + j * 512,
                                    ap=[[0, 64], [1, 512]],
                                )
                                nc.sync.dma_start(bc_sb, scr_bcast)
                                nc.vector.tensor_mul(
                                    ot_sb[po : po + 64, pr, j * 512 : (j + 1) * 512],
                                    o_ps[0:64, 0:512],
                                    bc_sb,
                                )

                    # ---- WO projection (psum via the shared "s" slots) ----
                    nech = d // 512
                    for it in range(tk):
                        y_sb = ysbp.tile([128, d], f32, tag="ysb")
                        for ec in range(nech):
                            y_ps = spsum.tile([128, 1024], f32, tag="s")
                            for dp in range(e // 128):
                                nc.tensor.matmul(
                                    y_ps[:, 0:512],
                                    lhsT=ot_sb[:, dp, it * 128 : (it + 1) * 128],
                                    rhs=wo_sb[:, dp, ec * 512 : (ec + 1) * 512],
                                    start=(dp == 0),
                                    stop=(dp == e // 128 - 1),
                                )
                            nc.vector.tensor_copy(
                                y_sb[:, ec * 512 : (ec + 1) * 512], y_ps[:, 0:512]
                            )
                        nc.sync.dma_start(y[it * 128 : (it + 1) * 128, :], y_sb)

    nc.compile()
    return nc


def _get_nc():
    if "nc" not in _cache:
        _cache["nc"] = _build()
    return _cache["nc"]


def _host_tables(positions):
    """cos/sin RoPE tables laid out for the on-chip [128, T] tiles."""
    pos = np.asarray(positions, np.float32)  # [t]
    inv = 1.0 / THETA ** (
        (2.0 * np.arange(1, DK // 2 + 1, dtype=np.float32) - 2.0) / DK
    )  # [32]
    ang = pos[None, :] * inv[:, None]  # [32, t]
    c32 = np.cos(ang)
    s32 = np.sin(ang)
    rows = np.arange(128)
    dloc = rows % DK
    fidx = dloc // 2
    sign = np.where(dloc % 2 == 0, -1.0, 1.0).astype(np.float32)
    cosT = c32[fidx, :]
    sinT = sign[:, None] * s32[fidx, :]
    return np.ascontiguousarray(cosT), np.ascontiguousarray(sinT)


def _make_in_maps(inputs):
    x = np.asarray(inputs["x"], np.float32)
    token_positions = np.asarray(inputs["token_positions"])
    WQ = np.asarray(inputs["WQ"], np.float32)
    WK = np.asarray(inputs["WK"], np.float32)
    WV = np.asarray(inputs["WV"], np.float32)
    WO = np.asarray(inputs["WO"], np.float32)
    trimask = np.triu(np.ones((128, 128), np.float32)).astype(_BF16)

    in_maps = []
    for c in range(NCORES):
        b, hg = c // 2, c % 2
        sl = slice(hg * E, (hg + 1) * E)
        cosT, sinT = _host_tables(token_positions[b])
        in_maps.append(
            {
                "xT": np.ascontiguousarray(x[b].T).astype(_BF16),
                "wqT": np.ascontiguousarray(WQ[sl, :].T).astype(_BF16),
                "wkT": np.ascontiguousarray(WK[sl, :].T).astype(_BF16),
                "wvT": np.ascontiguousarray(WV[sl, :].T).astype(_BF16),
                "woT": np.ascontiguousarray(WO[:, sl].T).astype(_BF16),
                "cosT": cosT,
                "sinT": sinT,
                "trim": trimask,
            }
        )
    return in_maps


def kernel(x, token_positions, WQ, WK, WV, WO):
    from concourse.bass_utils import run_bass_kernel_spmd

    nc = _get_nc()
    in_maps = _make_in_maps(
        {
            "x": x,
            "token_positions": token_positions,
            "WQ": WQ,
            "WK": WK,
            "WV": WV,
            "WO": WO,
        }
    )
    res = run_bass_kernel_spmd(nc, in_maps, core_ids=list(range(NCORES)))
    out = np.empty((B, T, D), np.float32)
    for b in range(B):
        out[b] = res.results[2 * b]["y"] + res.results[2 * b + 1]["y"]
    return out
